# revision 10
# baseline (speedup 1.0000x reference)
"""Single-head causal attention (B=8, T=2048, D=1024, H=128) on 8 TRN2 NeuronCores.

Sharding: one batch element per core (data-parallel over B).

Per-core algorithm (bf16 inputs, fp32 PSUM accumulation):
  - host supplies x^T [D, T] (partition-major [128, ND, T]) and weights bf16
  - Q^T, K^T = W^T @ x^T [H=128, T] via PE (f32 PSUM -> bf16 SBUF)
  - V^T likewise, PE-transposed to V [T, H] bf16 tiles (4 transposes ->
    one PSUM tile -> one evac)
  - per 512-wide q-chunk, k-tiles processed in PAIRS: two S^T matmuls land
    in one [128,1024] PSUM tile (2 banks), a single ACT exp converts the
    pair to bf16 P^T, causal masking only on the two 128x128 diagonal
    blocks via gpsimd affine_select (in-place), two PV matmuls accumulate
    O^T in PSUM, DVE accumulates the bf16 row-sum partials.
  - chunk tails (reciprocal_approx_fast -> gpsimd partition_broadcast ->
    DVE scale -> DMA out) are emitted after the next chunk's projections
    so they never stall the in-order PE queue (software pipelining).
  - constants/masks/ACT-table warm + PE warm-up matmuls run before the
    input DMA queue is built up, so nothing serializes behind the 5MB load.
  - host transposes O^T -> [T, H] per batch.
"""
import numpy as np

B, T, D, H = 8, 2048, 1024, 128
ND = D // 128      # 8 d-tiles
NTK = T // 128     # 16 k-tiles
NCH = T // 512     # 4 q-chunks
SCALE = float(H) ** -0.5

_CACHE = {}


def _build():
    import concourse.bass as bass  # noqa: F401
    from concourse import bacc
    import concourse.mybir as mybir
    import concourse.tile as tile
    from concourse.masks import make_identity

    f32 = mybir.dt.float32
    bf16 = mybir.dt.bfloat16

    nc = bacc.Bacc("TRN2", target_bir_lowering=False)
    # xt[p, n, t] = x[b].T[n*128 + p, t] — partition-major so a whole chunk
    # can stream with one 3D DMA
    xt_d = nc.dram_tensor("xt", (128, ND, T), bf16, kind="ExternalInput")
    wq_d = nc.dram_tensor("wq", (128, ND, H), bf16, kind="ExternalInput")
    wk_d = nc.dram_tensor("wk", (128, ND, H), bf16, kind="ExternalInput")
    wv_d = nc.dram_tensor("wv", (128, ND, H), bf16, kind="ExternalInput")
    ot_d = nc.dram_tensor("ot", (H, T), f32, kind="ExternalOutput")

    with tile.TileContext(nc) as tc:
        with (
            tc.tile_pool(name="sb", bufs=1) as sb,
            tc.tile_pool(name="ps", bufs=1, space="PSUM") as ps,
        ):
            # ---- constants first: the gpsimd library load + mask builds +
            # ACT table load all happen under the input-DMA shadow ----
            ident = sb.tile([128, 128], bf16, tag="ident")
            make_identity(nc, ident[:])
            # tri32[k, q] = 1 iff q >= k (same mask for every diagonal block)
            ones_c32 = sb.tile([128, 1], f32, tag="ones_c32")
            nc.gpsimd.memset(ones_c32[:], 1.0)
            ones_col = sb.tile([128, 1], bf16, tag="ones_col")
            nc.vector.tensor_copy(ones_col[:], ones_c32[:])
            # warm the ACT exp table while DMA streams in
            warm = sb.tile([128, 1], bf16, tag="warm")
            nc.scalar.activation(warm[:], ones_c32[:],
                                 mybir.ActivationFunctionType.Exp, scale=1.0)

            # ---- loads (weights first: LDWEIGHTS needs them earliest) ----
            wq = sb.tile([128, ND, H], bf16, tag="wq")
            wk = sb.tile([128, ND, H], bf16, tag="wk")
            wv = sb.tile([128, ND, H], bf16, tag="wv")
            nc.sync.dma_start(wv[:], wv_d[:])
            nc.sync.dma_start(wk[:], wk_d[:])
            nc.sync.dma_start(wq[:], wq_d[:])
            xt = sb.tile([128, ND, T], bf16, tag="xt")
            # chunk 0 at d-tile granularity (compute starts asap), rest coarse
            for d in range(ND):
                nc.sync.dma_start(xt[:, d, 0:512], xt_d[:, d, 0:512])
            for ch in range(1, NCH):
                nc.sync.dma_start(xt[:, :, ch * 512:(ch + 1) * 512],
                                  xt_d[:, :, ch * 512:(ch + 1) * 512])

            # warm the PE clock (HAM activity window) while DMA streams in
            wmm = ps.tile([128, 128], f32, tag="red")
            for i in range(16):
                nc.tensor.matmul(wmm[:], ident[:], ident[:],
                                 start=(i == 0), stop=(i == 15))

            qt = sb.tile([128, T], bf16, tag="qt")   # Q^T [h, t]
            kt = sb.tile([128, T], bf16, tag="kt")   # K^T [h, t]
            v = sb.tile([128, NTK, H], bf16, tag="v")  # V [k, h] tiles

            def body(c):
                """S/exp/PV/pacc over k-tile pairs + row sums for chunk c."""
                otp = ps.tile([128, 512], f32, tag="otacc", bufs=2)
                pacc = sb.tile([128, 512], bf16, tag="pacc", bufs=2)
                nk = 4 * c + 4
                first = True
                for j0 in range(0, nk, 2):
                    # pair (j0, j0+1); los[i] = valid q_local start of tile
                    los = []
                    for j in (j0, j0 + 1):
                        diag = j >= 4 * c
                        los.append(128 * (j - 4 * c) if diag else 0)
                    stp = ps.tile([128, 1024], f32, tag="big", bufs=2)
                    for i, j in enumerate((j0, j0 + 1)):
                        nc.tensor.matmul(
                            stp[:, 512 * i + los[i]:512 * (i + 1)],
                            kt[:, j * 128:(j + 1) * 128],
                            qt[:, c * 512 + los[i]:(c + 1) * 512],
                            start=True, stop=True,
                        )
                    pt = sb.tile([128, 1024], bf16, tag="pt", bufs=3)
                    # one exp for the pair (cols between the two valid spans
                    # hold garbage; nothing downstream reads them)
                    nc.scalar.activation(
                        pt[:, los[0]:1024], stp[:, los[0]:1024],
                        mybir.ActivationFunctionType.Exp, scale=SCALE)
                    for i, j in enumerate((j0, j0 + 1)):
                        if j >= 4 * c:  # diag: zero upper triangle in-place
                            colo = 512 * i + los[i]
                            nc.gpsimd.affine_select(
                                out=pt[:, colo:colo + 128],
                                in_=pt[:, colo:colo + 128],
                                compare_op=mybir.AluOpType.is_ge, fill=0.0,
                                base=0, pattern=[[1, 128]],
                                channel_multiplier=-1,
                            )
                        nc.tensor.matmul(
                            otp[:, los[i]:512], v[:, j, :],
                            pt[:, 512 * i + los[i]:512 * (i + 1)],
                            start=(j == 0), stop=(j == nk - 1),
                        )
                        with nc.allow_low_precision(reason="bf16 softmax denom"):
                            if first:
                                nc.vector.tensor_copy(pacc[:], pt[:, 0:512])
                                first = False
                            else:
                                nc.vector.tensor_add(
                                    pacc[:, los[i]:512], pacc[:, los[i]:512],
                                    pt[:, 512 * i + los[i]:512 * (i + 1)])
                sums = ps.tile([1, 512], f32, tag="red")
                nc.tensor.matmul(sums[:], ones_col[:], pacc[:], start=True, stop=True)
                return otp, sums

            def tail(c, otp, sums):
                """normalize + DMA out for chunk c (emitted late: overlaps
                the next chunk's projections, so nothing here stalls PE)."""
                recip = sb.tile([1, 512], f32, tag="recip", bufs=2)
                nc.vector.reciprocal_approx_fast(out=recip[:], in_=sums[:])
                bc_sb = sb.tile([128, 512], f32, tag="bcsb", bufs=2)
                nc.gpsimd.partition_broadcast(bc_sb[:], recip[:])
                ot_sb = sb.tile([128, 512], f32, tag="otsb", bufs=2)
                nc.vector.tensor_mul(ot_sb[:], otp[:], bc_sb[:])
                nc.sync.dma_start(ot_d[:, c * 512:(c + 1) * 512], ot_sb[:])

            # ---- chunk-major pipeline, tails deferred one chunk ----
            pend = None
            for ch in range(NCH):
                vt = sb.tile([128, 512], bf16, tag="vt", bufs=2)  # V^T staging
                for w_sb, dst in ((wv, vt), (wk, kt), (wq, qt)):
                    acc = ps.tile([128, 512], f32, tag="big", bufs=2,
                                  name=f"acc_{ch}")
                    for d in range(ND):
                        nc.tensor.matmul(
                            acc[:], w_sb[:, d, :],
                            xt[:, d, ch * 512:(ch + 1) * 512],
                            start=(d == 0), stop=(d == ND - 1),
                        )
                    with nc.allow_low_precision(reason="bf16 qkv"):
                        if dst is vt:
                            nc.scalar.copy(dst[:], acc[:])
                        else:
                            nc.vector.tensor_copy(dst[:, ch * 512:(ch + 1) * 512], acc[:])
                # 4 transposes -> one PSUM tile -> one evac
                tp = ps.tile([128, 512], bf16, tag="tp")
                for jj in range(4):
                    nc.tensor.transpose(tp[:, jj * 128:(jj + 1) * 128],
                                        vt[:, jj * 128:(jj + 1) * 128], ident[:])
                with nc.allow_low_precision(reason="bf16 v"):
                    nc.vector.tensor_copy(v[:, 4 * ch:4 * ch + 4, :], tp[:])
                if pend is not None:
                    tail(*pend)
                otp, sums = body(ch)
                pend = (ch, otp, sums)
            tail(*pend)

    nc.compile()
    return nc


def _in_maps(x, W_Q, W_K, W_V):
    import ml_dtypes

    bf16 = ml_dtypes.bfloat16

    def warr(W):
        return np.ascontiguousarray(
            np.asarray(W, np.float32).reshape(ND, 128, H).transpose(1, 0, 2)
        ).astype(bf16)

    wqr, wkr, wvr = warr(W_Q), warr(W_K), warr(W_V)
    x = np.asarray(x, np.float32)
    return [
        {"xt": np.ascontiguousarray(
            x[b].T.reshape(ND, 128, T).transpose(1, 0, 2)).astype(bf16),
         "wq": wqr, "wk": wkr, "wv": wvr}
        for b in range(B)
    ]


def _run(inputs, **kw):
    from concourse import bass_utils

    if "nc" not in _CACHE:
        _CACHE["nc"] = _build()
    return bass_utils.run_bass_kernel_spmd(
        _CACHE["nc"], _in_maps(**inputs), core_ids=list(range(B)), **kw)


def kernel(x, W_Q, W_K, W_V):
    res = _run({"x": x, "W_Q": W_Q, "W_K": W_K, "W_V": W_V})
    return np.stack([res.results[b]["ot"].T for b in range(B)]).astype(np.float32)


# revision 11
# speedup vs baseline: 1.2438x; 1.2438x over previous
"""Single-head causal attention (B=8, T=2048, D=1024, H=128) on 8 TRN2 NeuronCores.

Sharding: one batch element per core (data-parallel over B).

Per-core algorithm (bf16 inputs, fp32 PSUM accumulation):
  - host supplies x^T [D, T] (partition-major [128, ND, T]) and weights bf16
  - Q^T, K^T = W^T @ x^T [H=128, T] via PE (f32 PSUM -> bf16 SBUF)
  - V^T likewise, PE-transposed to V [T, H] bf16 tiles (4 transposes ->
    one PSUM tile -> one evac)
  - per 512-wide q-chunk: S^T[k,q] = K^T_tile.T @ Q^T_chunk (bf16), exp via
    ACT to bf16 P^T, causal mask on the 128x128 diagonal block via DVE
    multiply, O^T[h,q] += V_tile.T @ P^T in PSUM, DVE bf16 row-sum partials.
    The j-loop is software-pipelined: S_{j+1} is emitted before PV_j so the
    in-order PE queue never waits on exp_j.
  - chunk tails (reciprocal_approx_fast -> gpsimd partition_broadcast ->
    DVE scale -> DMA out) are emitted after the next chunk's projections.
  - constants/masks/ACT-table warm + PE warm-up matmuls run before the
    input DMA queue builds up; input DMA issues are split across the sync
    and scalar DGE rings.
  - host transposes O^T -> [T, H] per batch.
"""
import numpy as np

B, T, D, H = 8, 2048, 1024, 128
ND = D // 128      # 8 d-tiles
NTK = T // 128     # 16 k-tiles
NCH = T // 512     # 4 q-chunks
SCALE = float(H) ** -0.5

_CACHE = {}


def _build():
    import concourse.bass as bass  # noqa: F401
    from concourse import bacc
    import concourse.mybir as mybir
    import concourse.tile as tile
    from concourse.masks import make_identity

    f32 = mybir.dt.float32
    bf16 = mybir.dt.bfloat16

    nc = bacc.Bacc("TRN2", target_bir_lowering=False)
    # xt[p, n, t] = x[b].T[n*128 + p, t] — partition-major so a whole chunk
    # can stream with one 3D DMA
    xt_d = nc.dram_tensor("xt", (128, ND, T), bf16, kind="ExternalInput")
    wq_d = nc.dram_tensor("wq", (128, ND, H), bf16, kind="ExternalInput")
    wk_d = nc.dram_tensor("wk", (128, ND, H), bf16, kind="ExternalInput")
    wv_d = nc.dram_tensor("wv", (128, ND, H), bf16, kind="ExternalInput")
    ot_d = nc.dram_tensor("ot", (H, T), f32, kind="ExternalOutput")

    with tile.TileContext(nc) as tc:
        with (
            tc.tile_pool(name="sb", bufs=1) as sb,
            tc.tile_pool(name="ps", bufs=1, space="PSUM") as ps,
        ):
            # ---- constants first: the gpsimd library load + mask builds +
            # ACT table load all happen under the input-DMA shadow ----
            ident = sb.tile([128, 128], bf16, tag="ident")
            make_identity(nc, ident[:])
            # tri32[k, q] = 1 iff q >= k (same mask for every diagonal block)
            tri32 = sb.tile([128, 128], f32, tag="tri32")
            nc.gpsimd.memset(tri32[:], 1.0)
            nc.gpsimd.affine_select(
                out=tri32[:], in_=tri32[:],
                compare_op=mybir.AluOpType.is_ge, fill=0.0,
                base=0, pattern=[[1, 128]], channel_multiplier=-1,
            )
            trimask = sb.tile([128, 128], bf16, tag="trimask")
            nc.vector.tensor_copy(trimask[:], tri32[:])
            ones_c32 = sb.tile([128, 1], f32, tag="ones_c32")
            nc.gpsimd.memset(ones_c32[:], 1.0)
            ones_col = sb.tile([128, 1], bf16, tag="ones_col")
            nc.vector.tensor_copy(ones_col[:], ones_c32[:])
            # warm the ACT exp table while DMA streams in
            warm = sb.tile([128, 1], bf16, tag="warm")
            nc.scalar.activation(warm[:], ones_c32[:],
                                 mybir.ActivationFunctionType.Exp, scale=1.0)

            # ---- loads; issues split across the two HW-DGE rings ----
            wq = sb.tile([128, ND, H], bf16, tag="wq")
            wk = sb.tile([128, ND, H], bf16, tag="wk")
            wv = sb.tile([128, ND, H], bf16, tag="wv")
            xt = sb.tile([128, ND, T], bf16, tag="xt")
            nc.sync.dma_start(wv[:], wv_d[:])
            # chunk 0 at d-tile granularity (compute starts asap), rest coarse
            for d in range(ND):
                nc.sync.dma_start(xt[:, d, 0:512], xt_d[:, d, 0:512])
            nc.scalar.dma_start(wk[:], wk_d[:])
            nc.scalar.dma_start(wq[:], wq_d[:])
            for ch in range(1, NCH):
                nc.scalar.dma_start(xt[:, :, ch * 512:(ch + 1) * 512],
                                    xt_d[:, :, ch * 512:(ch + 1) * 512])

            # warm the PE clock (HAM activity window) while DMA streams in
            wmm = ps.tile([128, 128], f32, tag="red")
            for i in range(16):
                nc.tensor.matmul(wmm[:], ident[:], ident[:],
                                 start=(i == 0), stop=(i == 15))

            qt = sb.tile([128, T], bf16, tag="qt")   # Q^T [h, t]
            kt = sb.tile([128, T], bf16, tag="kt")   # K^T [h, t]
            v = sb.tile([128, NTK, H], bf16, tag="v")  # V [k, h] tiles

            def body(c):
                """Software-pipelined S/exp/PV/pacc loop + row sums."""
                otp = ps.tile([128, 512], f32, tag="otacc", bufs=2)
                pacc = sb.tile([128, 512], bf16, tag="pacc", bufs=2)
                nk = 4 * c + 4

                def lo_of(j):
                    return 128 * (j - 4 * c) if j >= 4 * c else 0

                def emit_s(j):
                    lo = lo_of(j)
                    stp = ps.tile([128, 512], f32, tag="big", bufs=4)
                    nc.tensor.matmul(
                        stp[:, lo:512],
                        kt[:, j * 128:(j + 1) * 128],
                        qt[:, c * 512 + lo:(c + 1) * 512],
                        start=True, stop=True,
                    )
                    pt = sb.tile([128, 512], bf16, tag="pt", bufs=6)
                    nc.scalar.activation(
                        pt[:, lo:512], stp[:, lo:512],
                        mybir.ActivationFunctionType.Exp, scale=SCALE)
                    if j >= 4 * c:  # diag: zero the upper-left triangle
                        nc.vector.tensor_mul(
                            pt[:, lo:lo + 128], pt[:, lo:lo + 128], trimask[:])
                    return pt

                def emit_pv(j, pt):
                    lo = lo_of(j)
                    nc.tensor.matmul(
                        otp[:, lo:512], v[:, j, :], pt[:, lo:512],
                        start=(j == 0), stop=(j == nk - 1),
                    )
                    with nc.allow_low_precision(reason="bf16 softmax denom"):
                        if j == 0:
                            nc.vector.tensor_copy(pacc[:], pt[:])
                        else:
                            nc.vector.tensor_add(pacc[:, lo:512], pacc[:, lo:512],
                                                 pt[:, lo:512])

                pts = {0: emit_s(0)}
                if nk > 1:
                    pts[1] = emit_s(1)
                for j in range(nk):
                    if j + 2 < nk:
                        pts[j + 2] = emit_s(j + 2)
                    emit_pv(j, pts.pop(j))
                sums = ps.tile([1, 512], f32, tag="red")
                nc.tensor.matmul(sums[:], ones_col[:], pacc[:], start=True, stop=True)
                return otp, sums

            def tail(c, otp, sums):
                """normalize + DMA out for chunk c (emitted late: overlaps
                the next chunk's projections, so nothing here stalls PE)."""
                recip = sb.tile([1, 512], f32, tag="recip", bufs=2)
                nc.vector.reciprocal_approx_fast(out=recip[:], in_=sums[:])
                bc_sb = sb.tile([128, 512], f32, tag="bcsb", bufs=2)
                nc.gpsimd.partition_broadcast(bc_sb[:], recip[:])
                ot_sb = sb.tile([128, 512], f32, tag="otsb", bufs=2)
                nc.vector.tensor_mul(ot_sb[:], otp[:], bc_sb[:])
                nc.sync.dma_start(ot_d[:, c * 512:(c + 1) * 512], ot_sb[:])

            # ---- chunk-major pipeline, tails deferred one chunk ----
            pend = None
            for ch in range(NCH):
                vt = sb.tile([128, 512], bf16, tag="vt", bufs=2)  # V^T staging
                for w_sb, dst in ((wv, vt), (wk, kt), (wq, qt)):
                    acc = ps.tile([128, 512], f32, tag="big", bufs=4,
                                  name=f"acc_{ch}")
                    for d in range(ND):
                        nc.tensor.matmul(
                            acc[:], w_sb[:, d, :],
                            xt[:, d, ch * 512:(ch + 1) * 512],
                            start=(d == 0), stop=(d == ND - 1),
                        )
                    with nc.allow_low_precision(reason="bf16 qkv"):
                        if dst is vt:
                            nc.vector.tensor_copy(dst[:], acc[:])
                        else:
                            nc.vector.tensor_copy(dst[:, ch * 512:(ch + 1) * 512], acc[:])
                # 4 transposes -> one PSUM tile -> one evac
                tp = ps.tile([128, 512], bf16, tag="tp")
                for jj in range(4):
                    nc.tensor.transpose(tp[:, jj * 128:(jj + 1) * 128],
                                        vt[:, jj * 128:(jj + 1) * 128], ident[:])
                with nc.allow_low_precision(reason="bf16 v"):
                    nc.vector.tensor_copy(v[:, 4 * ch:4 * ch + 4, :], tp[:])
                if pend is not None:
                    tail(*pend)
                otp, sums = body(ch)
                pend = (ch, otp, sums)
            tail(*pend)

    nc.compile()
    return nc


def _in_maps(x, W_Q, W_K, W_V):
    import ml_dtypes

    bf16 = ml_dtypes.bfloat16

    def warr(W):
        return np.ascontiguousarray(
            np.asarray(W, np.float32).reshape(ND, 128, H).transpose(1, 0, 2)
        ).astype(bf16)

    wqr, wkr, wvr = warr(W_Q), warr(W_K), warr(W_V)
    x = np.asarray(x, np.float32)
    return [
        {"xt": np.ascontiguousarray(
            x[b].T.reshape(ND, 128, T).transpose(1, 0, 2)).astype(bf16),
         "wq": wqr, "wk": wkr, "wv": wvr}
        for b in range(B)
    ]


def _run(inputs, **kw):
    from concourse import bass_utils

    if "nc" not in _CACHE:
        _CACHE["nc"] = _build()
    return bass_utils.run_bass_kernel_spmd(
        _CACHE["nc"], _in_maps(**inputs), core_ids=list(range(B)), **kw)


def kernel(x, W_Q, W_K, W_V):
    res = _run({"x": x, "W_Q": W_Q, "W_K": W_K, "W_V": W_V})
    return np.stack([res.results[b]["ot"].T for b in range(B)]).astype(np.float32)
